# revision 1
# baseline (speedup 1.0000x reference)
"""Multi-head attention (B=2, L=2048, E=1024, H=16) on 8 trn2 NeuronCores.

Sharding: core c -> (batch b = c//4, head-group g = c%4). Each core computes
4 heads (256 feature dims) of one batch: Q/K/V projections column-sliced,
full attention for its heads, and its slice of the output projection
(Wo row-parallel). Host sums the 4 partial products per batch and adds
bo + Wo@bv (the bv term commutes through softmax-normalized attention).

Device layout notes:
 - activations kept feature-on-partitions ("transposed"): qT/kT [256, L]
 - scores computed transposed: sT[k, q] = kT-slice.T @ qT-slice, so exp runs
   tile-local; the key-axis softmax sum comes free from a ones-column
   appended to V in the PV matmul (out row 64 = sum of exp).
 - softmax without max-subtraction: scores are O(1) here and masked entries
   are -1e9 -> exp underflows to exactly 0. Shift-invariance makes this
   mathematically identical to the reference.
 - matmuls run as float32r (full PE rate at moving-dim >= 256); fp32r inputs
   must be produced rounded, so DMA-loaded operands get rounding copies
   (placed on GPSIMD, which is otherwise idle).
 - attention processes key-blocks in pairs: one [128,2,512] scores psum, one
   masked add, one exp, two PV matmuls -- halves ACT/DVE instruction count.
 - output projection packs head pairs into the 128 partition rows so each
   matmul contracts K=128 (st/wo layout [128, 2, ...]).
"""

import numpy as np

B, L, E, H = 2, 2048, 1024, 16
Dh = 64
NCORES = 8
HPC = 4           # heads per core
DG = HPC * Dh     # 256 features per core
Dv = Dh + 1       # v width incl. ones column
NEG = np.float32(-1e9)

USE_F32R = True

_CACHE = {}
LAST_RESULTS = None


def _build(variant, Lb, use_f32r, reps=1):
    import concourse.bacc as bacc
    import concourse.tile as tile
    from concourse import mybir
    from contextlib import ExitStack

    f32 = mybir.dt.float32
    nq = Lb // 512         # q blocks
    nk = Lb // 128         # k blocks == l tiles
    ne = E // 128          # e-chunks (8)
    ndc = DG // 128        # feature chunks per core (2)

    nc = bacc.Bacc()
    xqT = nc.dram_tensor("xqT", [E, Lb], f32, kind="ExternalInput")
    xkT = nc.dram_tensor("xkT", [E, Lb], f32, kind="ExternalInput")
    xvT = nc.dram_tensor("xvT", [E, Lb], f32, kind="ExternalInput")
    wqT = nc.dram_tensor("wqT", [E, DG], f32, kind="ExternalInput")
    wkT = nc.dram_tensor("wkT", [E, DG], f32, kind="ExternalInput")
    wvT = nc.dram_tensor("wvT", [E, DG], f32, kind="ExternalInput")
    bqs = nc.dram_tensor("bqs", [128, ndc], f32, kind="ExternalInput")
    bks = nc.dram_tensor("bks", [128, ndc], f32, kind="ExternalInput")
    woT = nc.dram_tensor("woT", [128, ndc, E], f32, kind="ExternalInput")
    dmask = maskT = None
    if variant == "causal":
        dmask = nc.dram_tensor("dmask", [128, 4, 512], f32, kind="ExternalInput")
    elif variant == "general":
        maskT = nc.dram_tensor("maskT", [Lb, Lb], f32, kind="ExternalInput")
    yT = nc.dram_tensor("yT", [E, Lb], f32, kind="ExternalOutput")

    cdt = mybir.dt.float32r if use_f32r else f32
    Exp = mybir.ActivationFunctionType.Exp
    Ident = mybir.ActivationFunctionType.Identity
    Copy = mybir.ActivationFunctionType.Copy

    with tile.TileContext(nc) as tc, ExitStack() as ctx:
        persist = ctx.enter_context(tc.tile_pool(name="persist", bufs=1))
        qT_s = persist.tile([128, ndc, Lb], cdt, tag="qT")
        kT_s = persist.tile([128, ndc, Lb], cdt, tag="kT")
        v_s = persist.tile([128, nk, HPC, Dv], cdt, tag="v")
        wo_s = persist.tile([128, ndc, E], cdt, tag="wo")
        bq_s = persist.tile([128, ndc], f32, tag="bq")
        bk_s = persist.tile([128, ndc], f32, tag="bk")

        for _rep in range(reps):
            nc.sync.dma_start(out=bq_s, in_=bqs[:, :])
            nc.sync.dma_start(out=bk_s, in_=bks[:, :])
            nc.vector.memset(v_s[:, :, :, Dh:Dv].bitcast(f32), 1.0)

            # ---------------- projections ----------------
            with tc.tile_pool(name="wproj", bufs=1) as wpool, \
                 tc.tile_pool(name="stage", bufs=2) as stg, \
                 tc.tile_pool(name="xv_res", bufs=1) as xvr:

                def load_w(wT, wtag):
                    w_s = wpool.tile([128, ne, DG], cdt, tag=wtag, name=wtag)
                    for ec in range(ne):
                        if use_f32r:
                            wl = stg.tile([128, DG], f32, tag="wl", name="wl")
                            nc.scalar.dma_start(
                                out=wl, in_=wT[ec * 128:(ec + 1) * 128, :])
                            nc.gpsimd.tensor_copy(out=w_s[:, ec, :], in_=wl)
                        else:
                            nc.scalar.dma_start(
                                out=w_s[:, ec, :],
                                in_=wT[ec * 128:(ec + 1) * 128, :])
                    return w_s

                def load_x(pool, xT, ec, tag, eng, cpeng, ltag, lbufs=3,
                           bufs=None):
                    xt = pool.tile([128, Lb], cdt, tag=tag, name=tag, bufs=bufs)
                    if use_f32r:
                        xl = stg.tile([128, Lb], f32, tag=ltag,
                                      name=ltag, bufs=lbufs)
                        eng.dma_start(
                            out=xl, in_=xT[ec * 128:(ec + 1) * 128, :])
                        cpeng.tensor_copy(out=xt, in_=xl)
                    else:
                        eng.dma_start(
                            out=xt, in_=xT[ec * 128:(ec + 1) * 128, :])
                    return xt

                # weights first: small DMAs + Pool rounding copies; the q
                # stream's first chunks only wait on wq (fast start).
                w_all = {}
                for wT, wtag in ((wqT, "wq"), (wkT, "wk"), (wvT, "wv")):
                    w_all[wtag] = load_w(wT, wtag)

                # resident x_v chunks: DMAs triggered from the ACT queue
                # (doesn't block the SP-paced q/k streams), rounded on Pool.
                xv_ts = [load_x(xvr, xvT, ec, f"xv{ec}", nc.scalar,
                                nc.gpsimd, "xvl", lbufs=2)
                         for ec in range(ne)]

                # wo rounding copy (wo2 layout [128, 2, E]) - needed last
                if use_f32r:
                    wol = stg.tile([128, ndc, E], f32, tag="xvl", name="wol",
                                   bufs=2)
                    nc.scalar.dma_start(out=wol, in_=woT[:, :, :])
                    nc.gpsimd.tensor_copy(out=wo_s, in_=wol)
                else:
                    nc.scalar.dma_start(out=wo_s, in_=woT[:, :, :])

                # q/k: transposed-layout projections, x streamed e-chunk-outer
                for name, xT, wT, wtag, bias_t, scale, outT in (
                    ("q", xqT, wqT, "wq", bq_s, 0.125, qT_s),
                    ("k", xkT, wkT, "wk", bk_s, 1.0, kT_s),
                ):
                    w_s = w_all[wtag]
                    with tc.tile_pool(name=f"x_{name}", bufs=3) as xsp, \
                         tc.tile_pool(name=f"ps_{name}", bufs=1,
                                      space="PSUM") as pp:
                        pss = [pp.tile([128, 512], f32, tag=f"pj{i}",
                                       name=f"pj{i}") for i in range(ndc * nq)]
                        for ec in range(ne):
                            xt = load_x(xsp, xT, ec, "xs", nc.sync,
                                        nc.vector, "xsl", lbufs=2)
                            for dc in range(ndc):
                                for ln in range(nq):
                                    nc.tensor.matmul(
                                        out=pss[dc * nq + ln],
                                        lhsT=w_s[:, ec, dc * 128:(dc + 1) * 128],
                                        rhs=xt[:, ln * 512:(ln + 1) * 512],
                                        start=(ec == 0), stop=(ec == ne - 1),
                                    )
                        for dc in range(ndc):
                            for ln in range(nq):
                                nc.scalar.activation(
                                    out=outT[:, dc, ln * 512:(ln + 1) * 512],
                                    in_=pss[dc * nq + ln],
                                    func=Ident,
                                    bias=bias_t[:, dc:dc + 1],
                                    scale=scale,
                                )
                # v in normal [l, d] layout, resident x. Two e-chunk-outer
                # passes of 8 output blocks (8 half-bank accumulators) so the
                # matmuls consume x_v chunks incrementally as DMAs land,
                # instead of stalling on the last chunk per output block.
                wv_s = w_all["wv"]
                with tc.tile_pool(name="ps_v", bufs=1, space="PSUM") as pp:
                    half = nk // 2
                    for ph in range(2):
                        pvs = [pp.tile([128, DG], f32, tag=f"pv{i}",
                                       name=f"pv{i}") for i in range(half)]
                        for ec in range(ne):
                            for i in range(half):
                                lt = ph * half + i
                                nc.tensor.matmul(
                                    out=pvs[i],
                                    lhsT=xv_ts[ec][:, lt * 128:(lt + 1) * 128],
                                    rhs=wv_s[:, ec, :],
                                    start=(ec == 0), stop=(ec == ne - 1),
                                )
                        for i in range(half):
                            lt = ph * half + i
                            nc.scalar.activation(
                                out=v_s[:, lt, :, 0:Dh],
                                in_=pvs[i].rearrange("p (h d) -> p h d", h=HPC),
                                func=Copy,
                            )

            # ---------------- attention + output projection ----------------
            with tc.tile_pool(name="stp", bufs=1) as stp:
                st_s = stp.tile([128, ndc, nq, 512], cdt, tag="st", name="st")
                dm_s = None
                if variant == "causal":
                    dm_s = stp.tile([128, 4, 512], f32, tag="dm", name="dm")
                    nc.sync.dma_start(out=dm_s, in_=dmask[:, :, :])
                with tc.tile_pool(name="ps_att", bufs=2, space="PSUM") as sp, \
                     tc.tile_pool(name="ps_out", bufs=1, space="PSUM") as op, \
                     tc.tile_pool(name="pt", bufs=4) as ptp, \
                     tc.tile_pool(name="mk", bufs=3) as mkp, \
                     tc.tile_pool(name="nrm", bufs=4) as nrm, \
                     tc.tile_pool(name="drp", bufs=4, space="DRAM") as drp:
                    for qn in range(nq):
                        kmax = min(nk, 4 * qn + 4) if variant == "causal" else nk
                        ps_o = [op.tile([Dv, 512], f32, tag=f"po{h}",
                                        name=f"po{h}") for h in range(HPC)]
                        for kg in range(kmax // 2):
                            kc0 = 2 * kg
                            if variant == "general":
                                mkt = mkp.tile([128, 2, 512], f32, tag="mkt",
                                               name="mkt")
                                nc.sync.dma_start(
                                    out=mkt,
                                    in_=maskT[kc0 * 128:(kc0 + 2) * 128,
                                              qn * 512:(qn + 1) * 512]
                                    .rearrange("(b p) q -> p b q", b=2))
                            db0 = kc0 - 4 * qn   # in {0,2} on diagonal groups
                            for h in range(HPC):
                                pb = (h % 2) * 64
                                dc = h // 2
                                ps_s = sp.tile([128, 2, 512], f32, tag="pss",
                                               name="pss")
                                for i in (0, 1):
                                    nc.tensor.matmul(
                                        out=ps_s[:, i, :],
                                        lhsT=kT_s[pb:pb + 64, dc,
                                                  (kc0 + i) * 128:
                                                  (kc0 + i + 1) * 128],
                                        rhs=qT_s[pb:pb + 64, dc,
                                                 qn * 512:(qn + 1) * 512],
                                        start=True, stop=True,
                                    )
                                if variant == "general":
                                    nc.vector.tensor_add(out=ps_s, in0=ps_s,
                                                         in1=mkt)
                                elif variant == "causal" and 0 <= db0 <= 2:
                                    nc.vector.tensor_add(
                                        out=ps_s, in0=ps_s,
                                        in1=dm_s[:, db0:db0 + 2, :])
                                pt = ptp.tile([128, 2, 512], cdt, tag="pt",
                                              name="pt")
                                nc.scalar.activation(out=pt, in_=ps_s, func=Exp)
                                for i in (0, 1):
                                    nc.tensor.matmul(
                                        out=ps_o[h],
                                        lhsT=v_s[:, kc0 + i, h, :],
                                        rhs=pt[:, i, :],
                                        start=(kc0 + i == 0),
                                        stop=(kc0 + i == kmax - 1),
                                    )
                        for h in range(HPC):
                            pb = (h % 2) * 64
                            rec = nrm.tile([128, 512], f32, tag="rec",
                                           name="rec")
                            nc.vector.reciprocal(out=rec[Dh:Dh + 1, :],
                                                 in_=ps_o[h][Dh:Dh + 1, :])
                            dscr = drp.tile([1, 512], f32, tag="dscr",
                                            name="dscr")
                            nc.sync.dma_start(out=dscr, in_=rec[Dh:Dh + 1, :])
                            rb = nrm.tile([Dh, 512], f32, tag="rb", name="rb")
                            nc.sync.dma_start(
                                out=rb, in_=dscr.to_broadcast([Dh, 512]))
                            nc.vector.tensor_mul(
                                out=st_s[pb:pb + Dh, h // 2, qn, :],
                                in0=ps_o[h][0:Dh, :], in1=rb)

                with tc.tile_pool(name="ps_y", bufs=4, space="PSUM") as yp, \
                     tc.tile_pool(name="yst", bufs=3) as ys:
                    for jc in range(ne):
                        for ln in range(nq):
                            yps = yp.tile([128, 512], f32, tag="yps",
                                          name="yps")
                            for dc in range(ndc):
                                nc.tensor.matmul(
                                    out=yps,
                                    lhsT=wo_s[:, dc, jc * 128:(jc + 1) * 128],
                                    rhs=st_s[:, dc, ln, :],
                                    start=(dc == 0), stop=(dc == ndc - 1),
                                )
                            yt = ys.tile([128, 512], f32, tag="yt", name="yt")
                            nc.vector.tensor_copy(out=yt, in_=yps)
                            nc.sync.dma_start(
                                out=yT[jc * 128:(jc + 1) * 128,
                                       ln * 512:(ln + 1) * 512],
                                in_=yt)

    nc.finalize()
    return nc


def _get_nc(variant, Lb=L, reps=1):
    key = (variant, Lb, USE_F32R, reps)
    if key not in _CACHE:
        _CACHE[key] = _build(variant, Lb, USE_F32R, reps)
    return _CACHE[key]


def _detect_variant(mask):
    m2 = np.asarray(mask).reshape(mask.shape[-2], mask.shape[-1])
    m01 = (m2 != 0)
    if m01.all():
        return "none", m2
    if np.array_equal(m01, np.tril(np.ones(m2.shape, bool))):
        return "causal", m2
    return "general", m2


def _dmask_np():
    kl = np.arange(128)[:, None, None]
    db = np.arange(4)[None, :, None]
    ql = np.arange(512)[None, None, :]
    return np.where(db * 128 + kl > ql, NEG, np.float32(0)).astype(np.float32)


def _make_in_maps(x_q, x_k, x_v, m2, variant, Wq, bq, Wk, bk, Wv, Wo):
    in_maps = []
    for c in range(NCORES):
        b, g = divmod(c, HPC)
        gs = slice(g * DG, (g + 1) * DG)
        # wo2[p, hc, j] = Wo[j, g*DG + (2*hc + p//64)*Dh + p%64]
        wog = Wo[:, gs].T.reshape(HPC, Dh, E)
        wo2 = np.zeros((128, DG // 128, E), np.float32)
        for h in range(HPC):
            wo2[(h % 2) * Dh:(h % 2 + 1) * Dh, h // 2, :] = wog[h]
        im = {
            "xqT": np.ascontiguousarray(x_q[b].T),
            "xkT": np.ascontiguousarray(x_k[b].T),
            "xvT": np.ascontiguousarray(x_v[b].T),
            "wqT": np.ascontiguousarray(Wq[gs, :].T),
            "wkT": np.ascontiguousarray(Wk[gs, :].T),
            "wvT": np.ascontiguousarray(Wv[gs, :].T),
            "bqs": np.ascontiguousarray((bq[gs] / 8.0).reshape(2, 128).T),
            "bks": np.ascontiguousarray(bk[gs].reshape(2, 128).T),
            "woT": wo2,
        }
        if variant == "causal":
            im["dmask"] = _dmask_np()
        elif variant == "general":
            madd = np.where(m2 == 0, NEG, np.float32(0)).astype(np.float32)
            im["maskT"] = np.ascontiguousarray(madd.T)
        in_maps.append(im)
    return in_maps


def kernel(x_q, x_k, x_v, mask, Wq, bq, Wk, bk, Wv, bv, Wo, bo):
    global LAST_RESULTS
    from concourse.bass_utils import run_bass_kernel_spmd

    x_q = np.asarray(x_q, np.float32)
    x_k = np.asarray(x_k, np.float32)
    x_v = np.asarray(x_v, np.float32)
    Wq = np.asarray(Wq, np.float32)
    Wk = np.asarray(Wk, np.float32)
    Wv = np.asarray(Wv, np.float32)
    Wo = np.asarray(Wo, np.float32)
    bq = np.asarray(bq, np.float32)
    bk = np.asarray(bk, np.float32)
    bv = np.asarray(bv, np.float32)
    bo = np.asarray(bo, np.float32)

    variant, m2 = _detect_variant(mask)
    nc = _get_nc(variant)
    in_maps = _make_in_maps(x_q, x_k, x_v, m2, variant, Wq, bq, Wk, bk, Wv, Wo)

    res = run_bass_kernel_spmd(nc, in_maps, core_ids=list(range(NCORES)))
    LAST_RESULTS = res

    corr = (bo + Wo @ bv).astype(np.float32)
    y = np.empty((B, L, E), np.float32)
    for b in range(B):
        acc = res.results[HPC * b]["yT"].copy()
        for g in range(1, HPC):
            acc += res.results[HPC * b + g]["yT"]
        y[b] = acc.T + corr
    return y



# revision 21
# speedup vs baseline: 318.5756x; 318.5756x over previous
"""Multi-head attention (B=2, L=2048, E=1024, H=16) on 8 trn2 NeuronCores.

Sharding: core c -> (batch b = c//4, head-group g = c%4). Each core computes
4 heads (256 feature dims) of one batch: Q/K/V projections column-sliced,
full attention for its heads, and its slice of the output projection
(Wo row-parallel). Host sums the 4 partial products per batch and adds
bo + Wo@bv (the bv term commutes through softmax-normalized attention).

Device layout notes:
 - all matmul operands are bf16, cast on the HOST, so no on-chip rounding
   copies are needed and DMA traffic halves; PSUM accumulation stays fp32.
 - activations kept feature-on-partitions ("transposed"): qT/kT [256, L]
 - scores computed transposed: sT[k, q] = kT-slice.T @ qT-slice, so exp runs
   tile-local; the key-axis softmax sum comes free from a ones-column
   appended to V in the PV matmul (out row 64 = sum of exp).
 - softmax without max-subtraction: scores are O(1) here and masked entries
   are -1e9 -> exp underflows to exactly 0. Shift-invariance makes this
   mathematically identical to the reference.
 - head pairs (even/odd) sit at partition offsets 0/64; their score matmuls
   contract only 64 partitions and use distinct PE row groups, so emitting
   a pair's scores back-to-back lets the PE run them concurrently.
 - attention processes key-blocks in pairs: one [128,2,512] scores psum, one
   masked add, one exp, two PV matmuls -- halves ACT/DVE instruction count.
 - softmax 1/sum broadcast runs on the (otherwise idle) GpSimd engine via
   partition_broadcast instead of a DRAM round trip.
 - output projection packs head pairs into the 128 partition rows so each
   matmul contracts K=128 (st/wo layout [128, 2, ...]).
 - `hwloop` wraps the whole body in a hardware For_i loop: same program,
   trip-count-many executions; used for low-noise device-time measurement.
"""

import numpy as np

B, L, E, H = 2, 2048, 1024, 16
Dh = 64
NCORES = 8
HPC = 4           # heads per core
DG = HPC * Dh     # 256 features per core
Dv = Dh + 1       # v width incl. ones column
NEG = np.float32(-1e9)

_CACHE = {}
LAST_RESULTS = None


def _build(variant, Lb, reps=1, hwloop=None):
    import concourse.bacc as bacc
    import concourse.tile as tile
    from concourse import mybir
    from contextlib import ExitStack

    f32 = mybir.dt.float32
    bf16 = mybir.dt.bfloat16
    nq = Lb // 512         # q blocks
    nk = Lb // 128         # k blocks == l tiles
    ne = E // 128          # e-chunks (8)
    ndc = DG // 128        # feature chunks per core (2)

    nc = bacc.Bacc()
    xqT = nc.dram_tensor("xqT", [E, Lb], bf16, kind="ExternalInput")
    xkT = nc.dram_tensor("xkT", [E, Lb], bf16, kind="ExternalInput")
    xvT = nc.dram_tensor("xvT", [E, Lb], bf16, kind="ExternalInput")
    wqT = nc.dram_tensor("wqT", [E, DG], bf16, kind="ExternalInput")
    wkT = nc.dram_tensor("wkT", [E, DG], bf16, kind="ExternalInput")
    wvT = nc.dram_tensor("wvT", [E, DG], bf16, kind="ExternalInput")
    bqs = nc.dram_tensor("bqs", [128, ndc], f32, kind="ExternalInput")
    bks = nc.dram_tensor("bks", [128, ndc], f32, kind="ExternalInput")
    woT = nc.dram_tensor("woT", [128, ndc, E], bf16, kind="ExternalInput")
    dmask = maskT = None
    if variant == "causal":
        dmask = nc.dram_tensor("dmask", [128, 4, 512], f32, kind="ExternalInput")
    elif variant == "general":
        maskT = nc.dram_tensor("maskT", [Lb, Lb], f32, kind="ExternalInput")
    f16 = mybir.dt.float16
    yT = nc.dram_tensor("yT", [E, Lb], f16, kind="ExternalOutput")

    Exp = mybir.ActivationFunctionType.Exp
    Copy = mybir.ActivationFunctionType.Copy
    MUL = mybir.AluOpType.mult
    ADD = mybir.AluOpType.add

    with tile.TileContext(nc) as tc, ExitStack() as ctx:
        if hwloop is not None:
            ctx.enter_context(tc.For_i(0, hwloop))
        persist = ctx.enter_context(tc.tile_pool(name="persist", bufs=1))
        qT_s = persist.tile([128, ndc, Lb], bf16, tag="qT")
        kT_s = persist.tile([128, ndc, Lb], bf16, tag="kT")
        v_s = persist.tile([128, nk, HPC, Dv], bf16, tag="v")
        wo_s = persist.tile([128, ndc, E], bf16, tag="wo")
        bq_s = persist.tile([128, ndc], f32, tag="bq")
        bk_s = persist.tile([128, ndc], f32, tag="bk")
        warm = persist.tile([128, 1], f32, tag="warm")

        for _rep in range(reps):
            nc.sync.dma_start(out=bq_s, in_=bqs[:, :])
            nc.sync.dma_start(out=bk_s, in_=bks[:, :])
            nc.vector.memset(v_s[:, :, :, Dh:Dv], 1.0)
            # tiny exp to pull the ACT table load off the critical path
            nc.scalar.activation(out=warm, in_=bq_s[:, 0:1], func=Exp)

            # ---------------- projections ----------------
            with tc.tile_pool(name="wproj", bufs=1) as wpool, \
                 tc.tile_pool(name="xv_res", bufs=1) as xvr:

                # weights first on the scalar queue: the q stream's first
                # chunks only wait on wq (fast start).
                w_all = {}
                for wT, wtag in ((wqT, "wq"), (wkT, "wk"), (wvT, "wv")):
                    w_s = wpool.tile([128, ne, DG], bf16, tag=wtag, name=wtag)
                    nc.scalar.dma_start(
                        out=w_s, in_=wT.rearrange("(n p) d -> p n d", p=128))
                    w_all[wtag] = w_s

                xv_t = xvr.tile([128, ne, Lb], bf16, tag="xv", name="xv")

                # q/k: transposed-layout projections, x streamed e-chunk-outer.
                # x_v chunks ride the same sync queue interleaved with x_k so
                # they land by the end of the k projection without starving
                # the q stream. Bias + scale run on the (idle) DVE.
                for name, xT, wtag, bias_t, scale, outT in (
                    ("q", xqT, "wq", bq_s, 0.125, qT_s),
                    ("k", xkT, "wk", bk_s, 1.0, kT_s),
                ):
                    w_s = w_all[wtag]
                    with tc.tile_pool(name=f"x_{name}", bufs=5) as xsp, \
                         tc.tile_pool(name=f"ps_{name}", bufs=1,
                                      space="PSUM") as pp:
                        pss = [pp.tile([128, 512], f32, tag=f"pj{i}",
                                       name=f"pj{i}") for i in range(ndc * nq)]
                        for ec in range(ne):
                            xt = xsp.tile([128, Lb], bf16, tag="xs", name="xs")
                            nc.sync.dma_start(
                                out=xt, in_=xT[ec * 128:(ec + 1) * 128, :])
                            if name == "k":
                                nc.sync.dma_start(
                                    out=xv_t[:, ec, :],
                                    in_=xvT[ec * 128:(ec + 1) * 128, :])
                            for dc in range(ndc):
                                for ln in range(nq):
                                    nc.tensor.matmul(
                                        out=pss[dc * nq + ln],
                                        lhsT=w_s[:, ec, dc * 128:(dc + 1) * 128],
                                        rhs=xt[:, ln * 512:(ln + 1) * 512],
                                        start=(ec == 0), stop=(ec == ne - 1),
                                    )
                        for dc in range(ndc):
                            for ln in range(nq):
                                if scale != 1.0:
                                    nc.vector.tensor_scalar(
                                        out=outT[:, dc, ln * 512:(ln + 1) * 512],
                                        in0=pss[dc * nq + ln],
                                        scalar1=scale,
                                        scalar2=bias_t[:, dc:dc + 1],
                                        op0=MUL, op1=ADD,
                                    )
                                else:
                                    nc.vector.tensor_scalar_add(
                                        out=outT[:, dc, ln * 512:(ln + 1) * 512],
                                        in0=pss[dc * nq + ln],
                                        scalar1=bias_t[:, dc:dc + 1],
                                    )
                    if name == "k":
                        # wo is needed only by the output projection; emit
                        # its DMA behind the k/v x-chunks on the sync queue
                        # so it doesn't eat DMA bandwidth at kernel start.
                        nc.sync.dma_start(out=wo_s, in_=woT[:, :, :])
                # v in normal [l, d] layout from resident x_v. Two passes of
                # 8 half-bank accumulators.
                wv_s = w_all["wv"]
                with tc.tile_pool(name="ps_v", bufs=1, space="PSUM") as pp:
                    half = nk // 2
                    for ph in range(2):
                        pvs = [pp.tile([128, DG], f32, tag=f"pv{i}",
                                       name=f"pv{i}") for i in range(half)]
                        for ec in range(ne):
                            for i in range(half):
                                lt = ph * half + i
                                nc.tensor.matmul(
                                    out=pvs[i],
                                    lhsT=xv_t[:, ec, lt * 128:(lt + 1) * 128],
                                    rhs=wv_s[:, ec, :],
                                    start=(ec == 0), stop=(ec == ne - 1),
                                )
                        for i in range(half):
                            lt = ph * half + i
                            nc.vector.tensor_copy(
                                out=v_s[:, lt, :, 0:Dh],
                                in_=pvs[i].rearrange("p (h d) -> p h d", h=HPC),
                            )

            # ---------------- attention + output projection ----------------
            with tc.tile_pool(name="stp", bufs=1) as stp:
                st_s = stp.tile([128, ndc, nq, 512], bf16, tag="st", name="st")
                dm_s = None
                if variant == "causal":
                    dm_s = stp.tile([128, 4, 512], f32, tag="dm", name="dm")
                    nc.sync.dma_start(out=dm_s, in_=dmask[:, :, :])
                with tc.tile_pool(name="ps_att", bufs=3, space="PSUM") as sp, \
                     tc.tile_pool(name="ps_out", bufs=1, space="PSUM") as op, \
                     tc.tile_pool(name="pt", bufs=4) as ptp, \
                     tc.tile_pool(name="mk", bufs=3) as mkp, \
                     tc.tile_pool(name="nrm", bufs=4) as nrm:
                    for qn in range(nq):
                        kmax = min(nk, 4 * qn + 4) if variant == "causal" else nk
                        # sweep k once per head pair: only 2 PV accumulators
                        # live at a time, freeing PSUM for a deeper score
                        # pipeline (bufs=3).
                        for hp in range(HPC // 2):
                            hs = (2 * hp, 2 * hp + 1)
                            ps_o = {h: op.tile([Dv, 512], f32, tag=f"po{h % 2}",
                                               name=f"po{h}")
                                    for h in hs}
                            for kg in range(kmax // 2):
                                kc0 = 2 * kg
                                if variant == "general":
                                    mkt = mkp.tile([128, 2, 512], f32,
                                                   tag="mkt", name="mkt")
                                    nc.sync.dma_start(
                                        out=mkt,
                                        in_=maskT[kc0 * 128:(kc0 + 2) * 128,
                                                  qn * 512:(qn + 1) * 512]
                                        .rearrange("(b p) q -> p b q", b=2))
                                db0 = kc0 - 4 * qn  # {0,2} on diagonal groups
                                ps_p, pt_p = {}, {}
                                # both heads' score matmuls back-to-back:
                                # even/odd heads use PE row groups 0-1/2-3
                                # and can run concurrently.
                                for h in hs:
                                    pb = (h % 2) * 64
                                    dc = h // 2
                                    ps_s = sp.tile([128, 2, 512], f32,
                                                   tag="pss", name="pss")
                                    ps_p[h] = ps_s
                                    for i in (0, 1):
                                        nc.tensor.matmul(
                                            out=ps_s[:, i, :],
                                            lhsT=kT_s[pb:pb + 64, dc,
                                                      (kc0 + i) * 128:
                                                      (kc0 + i + 1) * 128],
                                            rhs=qT_s[pb:pb + 64, dc,
                                                     qn * 512:(qn + 1) * 512],
                                            start=True, stop=True,
                                        )
                                for h in hs:
                                    if variant == "general":
                                        nc.vector.tensor_add(
                                            out=ps_p[h], in0=ps_p[h], in1=mkt)
                                    elif variant == "causal" and 0 <= db0 <= 2:
                                        # only columns q < (db0+i+1)*128 can
                                        # be masked; trim the add to them
                                        for i in (0, 1):
                                            c = min(512, (db0 + i + 1) * 128)
                                            nc.vector.tensor_add(
                                                out=ps_p[h][:, i, 0:c],
                                                in0=ps_p[h][:, i, 0:c],
                                                in1=dm_s[:, db0 + i, 0:c])
                                for h in hs:
                                    pt = ptp.tile([128, 2, 512], bf16,
                                                  tag="pt", name="pt")
                                    pt_p[h] = pt
                                    nc.scalar.activation(out=pt, in_=ps_p[h],
                                                         func=Exp)
                                for h in hs:
                                    for i in (0, 1):
                                        nc.tensor.matmul(
                                            out=ps_o[h],
                                            lhsT=v_s[:, kc0 + i, h, :],
                                            rhs=pt_p[h][:, i, :],
                                            start=(kc0 + i == 0),
                                            stop=(kc0 + i == kmax - 1),
                                        )
                            # copy PSUM out fast (frees the banks for the
                            # next sweep's PV accumulation), normalize lazily.
                            stus, recs = {}, {}
                            for h in hs:
                                stu = nrm.tile([Dv, 512], f32, tag="stu",
                                               name="stu", bufs=8)
                                nc.vector.tensor_copy(out=stu, in_=ps_o[h])
                                stus[h] = stu
                            for h in hs:
                                rec = nrm.tile([1, 512], f32, tag="rec",
                                               name="rec", bufs=8)
                                nc.vector.reciprocal(out=rec,
                                                     in_=stus[h][Dh:Dh + 1, :])
                                recs[h] = rec
                            for h in hs:
                                rb = nrm.tile([Dh, 512], f32, tag="rb",
                                              name="rb", bufs=8)
                                nc.gpsimd.partition_broadcast(rb, recs[h])
                                pb = (h % 2) * 64
                                nc.vector.tensor_mul(
                                    out=st_s[pb:pb + Dh, h // 2, qn, :],
                                    in0=stus[h][0:Dh, :], in1=rb)

                with tc.tile_pool(name="ps_y", bufs=2, space="PSUM") as yp, \
                     tc.tile_pool(name="yst", bufs=3) as ys:
                    for jc in range(ne):
                        for l2 in range(nq // 2):
                            yps = yp.tile([128, 2, 512], f32, tag="yps",
                                          name="yps")
                            for i2 in range(2):
                                for dc in range(ndc):
                                    nc.tensor.matmul(
                                        out=yps[:, i2, :],
                                        lhsT=wo_s[:, dc,
                                                  jc * 128:(jc + 1) * 128],
                                        rhs=st_s[:, dc, 2 * l2 + i2, :],
                                        start=(dc == 0), stop=(dc == ndc - 1),
                                    )
                            yt = ys.tile([128, 1024], f16, tag="yt",
                                         name="yt")
                            if l2 % 2 == 0:
                                nc.vector.tensor_copy(
                                    out=yt.rearrange("p (b q) -> p b q", b=2),
                                    in_=yps)
                            else:
                                nc.scalar.activation(
                                    out=yt.rearrange("p (b q) -> p b q", b=2),
                                    in_=yps, func=Copy)
                            nc.sync.dma_start(
                                out=yT[jc * 128:(jc + 1) * 128,
                                       l2 * 1024:(l2 + 1) * 1024],
                                in_=yt)

    nc.finalize()
    return nc


def _get_nc(variant, Lb=L, reps=1, hwloop=None):
    key = (variant, Lb, reps, hwloop)
    if key not in _CACHE:
        _CACHE[key] = _build(variant, Lb, reps, hwloop=hwloop)
    return _CACHE[key]


def _detect_variant(mask):
    m2 = np.asarray(mask).reshape(mask.shape[-2], mask.shape[-1])
    m01 = (m2 != 0)
    if m01.all():
        return "none", m2
    if np.array_equal(m01, np.tril(np.ones(m2.shape, bool))):
        return "causal", m2
    return "general", m2


def _dmask_np():
    kl = np.arange(128)[:, None, None]
    db = np.arange(4)[None, :, None]
    ql = np.arange(512)[None, None, :]
    return np.where(db * 128 + kl > ql, NEG, np.float32(0)).astype(np.float32)


def _make_in_maps(x_q, x_k, x_v, m2, variant, Wq, bq, Wk, bk, Wv, Wo):
    import ml_dtypes
    bf16 = ml_dtypes.bfloat16
    in_maps = []
    # per-batch transposed activations, cast once and shared across cores
    xqTb = [np.ascontiguousarray(x_q[b].T).astype(bf16) for b in range(B)]
    xkTb = [np.ascontiguousarray(x_k[b].T).astype(bf16) for b in range(B)]
    xvTb = [np.ascontiguousarray(x_v[b].T).astype(bf16) for b in range(B)]
    dm = _dmask_np() if variant == "causal" else None
    madd = None
    if variant == "general":
        madd = np.ascontiguousarray(
            np.where(m2 == 0, NEG, np.float32(0)).astype(np.float32).T)
    for c in range(NCORES):
        b, g = divmod(c, HPC)
        gs = slice(g * DG, (g + 1) * DG)
        # wo2[p, hc, j] = Wo[j, g*DG + (2*hc + p//64)*Dh + p%64]
        wog = Wo[:, gs].T.reshape(HPC, Dh, E)
        wo2 = np.zeros((128, DG // 128, E), np.float32)
        for h in range(HPC):
            wo2[(h % 2) * Dh:(h % 2 + 1) * Dh, h // 2, :] = wog[h]
        im = {
            "xqT": xqTb[b],
            "xkT": xkTb[b],
            "xvT": xvTb[b],
            "wqT": np.ascontiguousarray(Wq[gs, :].T).astype(bf16),
            "wkT": np.ascontiguousarray(Wk[gs, :].T).astype(bf16),
            "wvT": np.ascontiguousarray(Wv[gs, :].T).astype(bf16),
            "bqs": np.ascontiguousarray((bq[gs] / 8.0).reshape(2, 128).T),
            "bks": np.ascontiguousarray(bk[gs].reshape(2, 128).T),
            "woT": wo2.astype(bf16),
        }
        if variant == "causal":
            im["dmask"] = dm
        elif variant == "general":
            im["maskT"] = madd
        in_maps.append(im)
    return in_maps


def kernel(x_q, x_k, x_v, mask, Wq, bq, Wk, bk, Wv, bv, Wo, bo):
    global LAST_RESULTS
    from concourse.bass_utils import run_bass_kernel_spmd

    x_q = np.asarray(x_q, np.float32)
    x_k = np.asarray(x_k, np.float32)
    x_v = np.asarray(x_v, np.float32)
    Wq = np.asarray(Wq, np.float32)
    Wk = np.asarray(Wk, np.float32)
    Wv = np.asarray(Wv, np.float32)
    Wo = np.asarray(Wo, np.float32)
    bq = np.asarray(bq, np.float32)
    bk = np.asarray(bk, np.float32)
    bv = np.asarray(bv, np.float32)
    bo = np.asarray(bo, np.float32)

    variant, m2 = _detect_variant(mask)
    nc = _get_nc(variant)
    in_maps = _make_in_maps(x_q, x_k, x_v, m2, variant, Wq, bq, Wk, bk, Wv, Wo)

    res = run_bass_kernel_spmd(nc, in_maps, core_ids=list(range(NCORES)))
    LAST_RESULTS = res

    corr = (bo + Wo @ bv).astype(np.float32)
    y = np.empty((B, L, E), np.float32)
    for b in range(B):
        acc = res.results[HPC * b]["yT"].astype(np.float32)
        for g in range(1, HPC):
            acc += res.results[HPC * b + g]["yT"].astype(np.float32)
        y[b] = acc.T + corr
    return y
